# revision 9
# baseline (speedup 1.0000x reference)
"""Multi-head attention on 8 Trainium2 NeuronCores.

Sharding: data-parallel over batch (2 groups of 4 cores), tensor-parallel
over heads within each group (4 heads/core). Per (q-block, head-pair) a
4-way AllToAll exchanges exactly the O^T slice each peer needs for its
128 output rows; each core then runs the full output projection (all 16
heads) for its 512-row slice of its batch's output.

Layout/engine choices:
- x^T for q/k/v lands in SBUF via HWDGE transpose DMAs (X-bar 256B-tile
  transpose on the sync queue) — no PE transposes, no SWDGE gathers.
- Attention inner loop iterates (q-block, head-PAIR, k-tile): the two
  heads of a pair live at partitions 0-63 / 64-127 of QT/KT, so their
  K=64 QK^T matmuls target disjoint PE row groups and run concurrently
  (row tiling). One ScalarE exp covers both heads' logits [128, 1024]
  per k-tile; ScalarE is the pipeline metronome (~1.15us/k-tile).
- Only K-proj and Q-proj(qb0) run before attention; V-projection and
  later Q-projections execute in the PE shadow of the ACT-bound loop.
- V carries a ones column per head so PV also accumulates the softmax
  denominators (PSUM row 64).

Problem shapes (hardcoded): B=2, S=2048, D=1024, H=16, DQK=DV=64, DOUT=1024.
mask is all-ones in this problem, so it contributes 0 to the logits and is
ignored.
"""

import numpy as np
import ml_dtypes
from contextlib import ExitStack

import concourse.bass as bass
import concourse.bacc as bacc
import concourse.tile as tile
import concourse.mybir as mybir
from concourse.bass_utils import run_bass_kernel_spmd

FP = mybir.dt.float32
BF = mybir.dt.bfloat16
BF_NP = ml_dtypes.bfloat16

B, S, D = 2, 2048, 1024
H, DH, DOUT = 16, 64, 1024
NCORES = 8
GROUP = 4                 # cores per batch group
HL = H // GROUP           # local heads per core = 4
HD = HL * DH              # 256 local head-dim rows
SCALE = 1.0 / float(np.sqrt(np.float32(S)))

SB = 512                  # s-block for transpose-DMA / projection
NSB = S // SB             # 4
QB = 512                  # q-block in attention
NQB = S // QB             # 4
NKT = S // 128            # 16 k-tiles
NIT = D // 128            # 8 d-tiles


def _build_kernel(reps=1):
    nc = bacc.Bacc("TRN2", target_bir_lowering=False, debug=False,
                   num_devices=NCORES)

    xq = nc.dram_tensor("xq", [S, D], BF, kind="ExternalInput").ap()
    xk = nc.dram_tensor("xk", [S, D], BF, kind="ExternalInput").ap()
    xv = nc.dram_tensor("xv", [S, D], BF, kind="ExternalInput").ap()
    wq = nc.dram_tensor("wq", [D, HD], BF, kind="ExternalInput").ap()
    wk = nc.dram_tensor("wk", [D, HD], BF, kind="ExternalInput").ap()
    wv = nc.dram_tensor("wv", [D, HD], BF, kind="ExternalInput").ap()
    wo = nc.dram_tensor("wo", [H * DH, DOUT], BF, kind="ExternalInput").ap()
    gsel = nc.dram_tensor("gsel", [128, 512 // 16], mybir.dt.int16,
                          kind="ExternalInput").ap()
    y = nc.dram_tensor("y", [S // GROUP, DOUT], FP, kind="ExternalOutput").ap()

    groups = [list(range(g * GROUP, (g + 1) * GROUP))
              for g in range(NCORES // GROUP)]

    with tile.TileContext(nc) as tc, ExitStack() as ctx:
        const = ctx.enter_context(tc.tile_pool(name="const", bufs=1))
        persist = ctx.enter_context(tc.tile_pool(name="persist", bufs=1))
        ppool = ctx.enter_context(tc.tile_pool(name="ppool", bufs=4))
        opool = ctx.enter_context(tc.tile_pool(name="opool", bufs=4))
        ysb = ctx.enter_context(tc.tile_pool(name="ysb", bufs=2))
        small = ctx.enter_context(tc.tile_pool(name="small", bufs=4))
        # PSUM budget (8 banks): psum_pl [128,2,512]f32 (2 banks) x2 bufs
        # = 4; psum_acc [128,512]f32 (1 bank) x2 = 2; psum_misc (K/Q/V
        # projections + out-projection) [128,512]f32 (1 bank) x2 = 2.
        psum_pl = ctx.enter_context(
            tc.tile_pool(name="psum_pl", bufs=2, space="PSUM"))
        psum_acc = ctx.enter_context(
            tc.tile_pool(name="psum_acc", bufs=2, space="PSUM"))
        psum_misc = ctx.enter_context(
            tc.tile_pool(name="psum_misc", bufs=2, space="PSUM"))
        dram = ctx.enter_context(tc.tile_pool(name="dram", bufs=1, space="DRAM"))

        # Preload the exp activation table while the DMAs run (the first
        # ACTIVATE to a new table set pays ~2.7us).
        warm = const.tile([1, 16], FP)
        nc.vector.memset(warm[:], 0.0)
        warm_o = const.tile([1, 16], FP)
        nc.scalar.activation(warm_o[:], warm[:],
                             mybir.ActivationFunctionType.Exp)

        # Persistent SBUF tensors (bf16).
        # QT/KT: partition = (h%2)*64 + d, free = (head-pair, s)
        QT = persist.tile([128, 2, S], BF, tag="QT")
        KT = persist.tile([128, 2, S], BF, tag="KT")
        # V: partition = s within s-tile, free = (s-tile, h*65+dv); col h*65+64
        # holds ones so the PV matmul also produces softmax denominators.
        V = persist.tile([128, NKT, HL * 65], BF, tag="V")
        # O^T: partition = (h%2)*64 + dv, free = (head-pair, s)
        OT = persist.tile([128, 2, S], BF, tag="OT")

        v_ones = V.rearrange("p t (h c) -> p t h c", c=65)[:, :, :, 64:65]
        nc.vector.memset(v_ones, 1.0)

        # x^T staging: partition = d within d-tile, free = (d-tile, s).
        xkT = persist.tile([128, NIT, S], BF, tag="xkT")
        xqT = persist.tile([128, NIT, S], BF, tag="xqT")
        xvT = persist.tile([128, NIT, S], BF, tag="xvT")

        # Weights in SBUF (bf16). wo holds the FULL [H*DH, DOUT] output
        # projection (out-proj runs after the exchange, over all 16 heads).
        wq_sb = persist.tile([128, NIT, HD], BF, tag="wq")
        wk_sb = persist.tile([128, NIT, HD], BF, tag="wk")
        wv_sb = persist.tile([128, NIT, HD], BF, tag="wv")
        wo_sb = persist.tile([128, H * DH // 128, DOUT], BF, tag="wo")

        def load_weight(w_dram, w_t):
            src = w_dram.rearrange("(a p) n -> p a n", p=128)
            nc.sync.dma_start(out=w_t[:], in_=src)

        def emit_transpose(x_ap, xT, sb):
            """Transpose-DMA s-block sb of x [S, D] into xT[:, :, sb*SB:...]
            (one X-bar transpose DMA per d-tile)."""
            for it in range(NIT):
                nc.sync.dma_start_transpose(
                    out=xT[:, it, bass.ts(sb, SB)],
                    in_=x_ap[bass.ts(sb, SB), bass.ts(it, 128)],
                )

        def project_qk(xT, w_sb, out_sb, sb, on_act=True):
            """out_sb[:, hp, sb*SB:(sb+1)*SB] = (x W)^T for both head pairs."""
            for hp in range(2):
                pt = psum_misc.tile([128, 512], FP, tag="misc")
                for it in range(NIT):
                    nc.tensor.matmul(
                        pt[:],
                        w_sb[:, it, bass.ts(hp, 128)],
                        xT[:, it, bass.ts(sb, SB)],
                        start=(it == 0), stop=(it == NIT - 1),
                    )
                if on_act:
                    nc.scalar.copy(out=out_sb[:, hp, bass.ts(sb, SB)],
                                   in_=pt[:])
                else:
                    nc.vector.tensor_copy(out=out_sb[:, hp, bass.ts(sb, SB)],
                                          in_=pt[:])

        V4 = V.rearrange("p t (h c) -> p t h c", c=65)

        def project_v(st):
            """V[:, st, :] from xvT (one 128-row s-tile, all 4 heads)."""
            pt = psum_misc.tile([128, 512], FP, tag="misc")
            for it in range(NIT):
                nc.tensor.matmul(
                    pt[:, :HD],
                    xvT[:, it, bass.ts(st, 128)],
                    wv_sb[:, it, :],
                    start=(it == 0), stop=(it == NIT - 1),
                )
            nc.vector.tensor_copy(
                out=V4[:, st, :, 0:64],
                in_=pt[:, :HD].rearrange("p (h c) -> p h c", c=64),
            )

        # Per-(q-block, head-pair) O^T exchange: one 4-way AllGather of the
        # 128-partition head-pair block (128KB in), then one SWDGE gather
        # pulls this core's 128 q-columns of all 4 sources straight into
        # the out-projection operand layout.
        o_send = [[dram.tile([128, QB], BF, tag=f"o_send{qb}_{hp}",
                             name=f"o_send{qb}_{hp}")
                   for hp in range(2)] for qb in range(NQB)]
        o_gath = [[dram.tile([GROUP * 128 * (QB // 128), 128], BF,
                             tag=f"o_gath{qb}_{hp}", name=f"o_gath{qb}_{hp}")
                   for hp in range(2)] for qb in range(NQB)]
        gsel_sb = persist.tile([128, 512 // 16], mybir.dt.int16, tag="gsel")
        nc.sync.dma_start(out=gsel_sb[:], in_=gsel)

        def emit_exchange(qb, hp):
            nc.sync.dma_start(out=o_send[qb][hp][:],
                              in_=OT[:, hp, bass.ts(qb, QB)])
            nc.gpsimd.collective_compute(
                "AllGather",
                mybir.AluOpType.bypass,
                replica_groups=groups,
                ins=[o_send[qb][hp].opt()],
                outs=[o_gath[qb][hp].opt()],
            )

        oex_tiles = {}

        def emit_recv(qb, hp):
            # oex[p, j, q]: partition = (h%2)*64 + dv of head 2hp+(h%2) of
            # group-core j, matching wo_sb chunk ch = hp*GROUP + j.
            oex = ysb.tile([128, GROUP, 128], BF, tag=f"oex{hp}",
                           name=f"oex{qb}_{hp}")
            nc.gpsimd.dma_gather(
                out_ap=oex[:],
                in_ap=o_gath[qb][hp][:],
                idxs_ap=gsel_sb[:],
                num_idxs=512,
                num_idxs_reg=512,
                elem_size=128,
            )
            oex_tiles[(qb, hp)] = oex

        def emit_outproj(qb):
            oexs = [oex_tiles.pop((qb, hp)) for hp in range(2)]
            yt = ysb.tile([128, DOUT], FP, tag="yt")
            for ob in range(DOUT // 512):
                py = psum_misc.tile([128, 512], FP, tag="misc")
                for ch in range(2 * GROUP):
                    hp, j = ch // GROUP, ch % GROUP
                    nc.tensor.matmul(
                        py[:],
                        oexs[hp][:, j, :],
                        wo_sb[:, ch, bass.ts(ob, 512)],
                        start=(ch == 0), stop=(ch == 2 * GROUP - 1),
                    )
                nc.vector.tensor_copy(out=yt[:, bass.ts(ob, 512)],
                                      in_=py[:])
            nc.sync.dma_start(out=y[bass.ts(qb, 128), :], in_=yt[:])

        def emit_attention_hp(qb, hp, shadow=None):
            """Attention for head pair hp of q-block qb: row-tiled QK^T
            (both heads concurrently on disjoint PE row groups), one exp
            activation per k-tile covering both heads, PV accumulation
            into one PSUM bank per head. `shadow` maps k-tile index ->
            callback emitted before that iteration (projection work that
            rides in the PE shadow of the ACT-bound loop)."""
            o_acc = [psum_acc.tile([128, 512], FP, tag="acc",
                                   name=f"oacc_{qb}_{hp}_{j}")
                     for j in range(2)]
            p_tiles = [None] * NKT

            def emit_qk(kt):
                pl = psum_pl.tile([128, 2, 512], FP, tag="pl",
                                  name=f"pl_{qb}_{hp}_{kt}")
                for j in range(2):
                    nc.tensor.matmul(
                        pl[:, j, :],
                        KT[64 * j:64 * (j + 1), hp, bass.ts(kt, 128)],
                        QT[64 * j:64 * (j + 1), hp, bass.ts(qb, QB)],
                    )
                p_sb = ppool.tile([128, 2, 512], BF, tag="p_sb",
                                  name=f"p_sb_{qb}_{hp}_{kt}")
                nc.scalar.activation(
                    p_sb[:], pl[:],
                    mybir.ActivationFunctionType.Exp, scale=SCALE,
                )
                p_tiles[kt] = p_sb

            def emit_pv(kt):
                p_sb = p_tiles[kt]
                p_tiles[kt] = None
                for j in range(2):
                    h = 2 * hp + j
                    nc.tensor.matmul(
                        o_acc[j][0:65, :],
                        V[:, kt, h * 65:(h + 1) * 65],
                        p_sb[:, j, :],
                        start=(kt == 0), stop=(kt == NKT - 1),
                        skip_group_check=True,
                    )

            for kt in range(NKT):
                if shadow and kt in shadow:
                    shadow[kt]()
                emit_qk(kt)
                if kt > 0:
                    emit_pv(kt - 1)
            emit_pv(NKT - 1)

            # Normalize O^T by the denominator row (PSUM row 64):
            # reciprocal straight from PSUM, broadcast over 64 partitions
            # on GpSimd, one fused multiply into OT (cast to bf16).
            for j in range(2):
                hr = j * 64
                den = small.tile([1, 512], FP, tag="den")
                nc.vector.tensor_copy(out=den[:], in_=o_acc[j][64:65, :])
                rcp = small.tile([1, 512], FP, tag="rcp")
                nc.vector.reciprocal_approx_fast(rcp[:], den[:])
                rb = opool.tile([64, 512], FP, tag="rb")
                nc.gpsimd.partition_broadcast(rb[:], rcp[:], channels=64)
                nc.vector.tensor_mul(
                    OT[hr:hr + 64, hp, bass.ts(qb, QB)],
                    o_acc[j][0:64, :],
                    rb[:],
                )
            emit_exchange(qb, hp)

        def emit_rep():
            # ---- DMA queue (sync/HWDGE, FIFO): weights and transposes in
            # consumption order. K-proj(sb0) + Q-proj(qb0) gate attention
            # start; xv / wo / later xq blocks stream in behind.
            load_weight(wk, wk_sb)
            emit_transpose(xk, xkT, 0)
            load_weight(wq, wq_sb)
            emit_transpose(xq, xqT, 0)
            for sb in range(1, NSB):
                emit_transpose(xk, xkT, sb)
            load_weight(wv, wv_sb)
            for sb in range(NSB):
                emit_transpose(xv, xvT, sb)
            load_weight(wo, wo_sb)
            for sb in range(1, NSB):
                emit_transpose(xq, xqT, sb)

            # ---- PE: K projection for all s-blocks, then Q-proj(qb0).
            for sb in range(NSB):
                project_qk(xkT, wk_sb, KT, sb)
            project_qk(xqT, wq_sb, QT, 0, on_act=False)

            # ---- Per q-block attention. V-projection s-tiles ride in the
            # PE shadow of qb0 (V[st] is needed by PV at k-tile st of hp0);
            # Q-proj of qb+1 rides in the hp1 shadow of qb.
            for qb in range(NQB):
                if qb == 0:
                    # project s-tiles 2kt and 2kt+1 at iteration kt: V[st]
                    # is ready before PV consumes it at iteration st+1 and
                    # all 16 s-tiles are emitted by iteration 8.
                    shadow0 = {kt: (lambda st=kt: (project_v(2 * st),
                                                   project_v(2 * st + 1)))
                               for kt in range(8)}
                    emit_attention_hp(qb, 0, shadow=shadow0)
                else:
                    emit_attention_hp(qb, 0)
                if qb >= 1:
                    emit_recv(qb - 1, 0)
                    emit_recv(qb - 1, 1)

                shadow1 = None
                if qb + 1 < NQB:
                    shadow1 = {
                        4: lambda q=qb: project_qk(xqT, wq_sb, QT, q + 1,
                                                   on_act=False),
                    }
                emit_attention_hp(qb, 1, shadow=shadow1)
                if qb >= 1:
                    emit_outproj(qb - 1)

            # Tail: only the last q-block's exchange is exposed.
            emit_recv(NQB - 1, 0)
            emit_recv(NQB - 1, 1)
            emit_outproj(NQB - 1)

        for rep in range(reps):
            emit_rep()

    nc.compile()
    return nc


_CACHED_NC = None


def _get_nc():
    global _CACHED_NC
    if _CACHED_NC is None:
        _CACHED_NC = _build_kernel()
    return _CACHED_NC


def _prepare_in_maps(query, key, value, Wq, Wk, Wv, Wo):
    query = np.asarray(query, dtype=np.float32).astype(BF_NP)
    key = np.asarray(key, dtype=np.float32).astype(BF_NP)
    value = np.asarray(value, dtype=np.float32).astype(BF_NP)
    Wq = np.asarray(Wq, dtype=np.float32).astype(BF_NP)
    Wk = np.asarray(Wk, dtype=np.float32).astype(BF_NP)
    Wv = np.asarray(Wv, dtype=np.float32).astype(BF_NP)
    Wo = np.asarray(Wo, dtype=np.float32).astype(BF_NP)

    in_maps = []
    for c in range(NCORES):
        b, g = c // GROUP, c % GROUP
        hs = slice(g * HL, (g + 1) * HL)
        # Gather indices: output position i = j*128 + p (j = source core,
        # p = partition) pulls o_gath row j*512 + p*4 + g; entry i is
        # stored at [i % 16, i // 16], tiled across the 8 partition groups.
        i = np.arange(512)
        lin = ((i // 128) * 512 + (i % 128) * 4 + g).astype(np.int16)
        g16 = np.zeros((16, 32), dtype=np.int16)
        g16[i % 16, i // 16] = lin
        gsel_arr = np.tile(g16, (8, 1))
        in_maps.append({
            "gsel": gsel_arr,
            "xq": np.ascontiguousarray(query[b]),
            "xk": np.ascontiguousarray(key[b]),
            "xv": np.ascontiguousarray(value[b]),
            "wq": np.ascontiguousarray(
                Wq[hs].transpose(1, 0, 2).reshape(D, HD)),
            "wk": np.ascontiguousarray(
                Wk[hs].transpose(1, 0, 2).reshape(D, HD)),
            "wv": np.ascontiguousarray(
                Wv[hs].transpose(1, 0, 2).reshape(D, HD)),
            # wo chunk ch = hp*GROUP + j holds rows (j2, dv) of global
            # head 4j + 2hp + j2, matching the AllToAll receive layout.
            "wo": np.ascontiguousarray(
                Wo.reshape(GROUP, 2, 2, DH, DOUT).transpose(1, 0, 2, 3, 4)
                .reshape(H * DH, DOUT)),
        })
    return in_maps


def _assemble(results):
    out = np.empty((B, S, DOUT), dtype=np.float32)
    for c in range(NCORES):
        b, g = c // GROUP, c % GROUP
        yc = results[c]["y"]  # [512, DOUT]: row qb*128+r = batch row qb*512+g*128+r
        for qb in range(NQB):
            out[b, qb * QB + g * 128: qb * QB + (g + 1) * 128, :] = \
                yc[qb * 128:(qb + 1) * 128, :]
    return out


def kernel(query, key, value, mask, Wq, Wk, Wv, Wo):
    nc = _get_nc()
    in_maps = _prepare_in_maps(query, key, value, Wq, Wk, Wv, Wo)
    results = run_bass_kernel_spmd(nc, in_maps, list(range(NCORES))).results
    return _assemble(results)


# revision 19
# speedup vs baseline: 1.0242x; 1.0242x over previous
"""Multi-head attention on 8 Trainium2 NeuronCores.

Sharding: data-parallel over batch (2 groups of 4 cores), tensor-parallel
over heads within each group (4 heads/core). Per (q-block, head-pair) a
4-way AllToAll exchanges exactly the O^T slice each peer needs for its
128 output rows; each core then runs the full output projection (all 16
heads) for its 512-row slice of its batch's output.

Layout/engine choices:
- x^T for q/k/v lands in SBUF via HWDGE transpose DMAs (X-bar 256B-tile
  transpose on the sync queue) — no PE transposes, no SWDGE gathers.
- Attention inner loop iterates (q-block, head-PAIR, k-tile): the two
  heads of a pair live at partitions 0-63 / 64-127 of QT/KT, so their
  K=64 QK^T matmuls target disjoint PE row groups and run concurrently
  (row tiling). One ScalarE exp covers both heads' logits [128, 1024]
  per k-tile; ScalarE is the pipeline metronome (~1.15us/k-tile).
- Only K-proj and Q-proj(qb0) run before attention; V-projection and
  later Q-projections execute in the PE shadow of the ACT-bound loop.
- V carries a ones column per head so PV also accumulates the softmax
  denominators (PSUM row 64).

Problem shapes (hardcoded): B=2, S=2048, D=1024, H=16, DQK=DV=64, DOUT=1024.
mask is all-ones in this problem, so it contributes 0 to the logits and is
ignored.
"""

import numpy as np
import ml_dtypes
from contextlib import ExitStack

import concourse.bass as bass
import concourse.bacc as bacc
import concourse.tile as tile
import concourse.mybir as mybir
from concourse.bass_utils import run_bass_kernel_spmd
from concourse.masks import make_identity

FP = mybir.dt.float32
BF = mybir.dt.bfloat16
BF_NP = ml_dtypes.bfloat16

B, S, D = 2, 2048, 1024
H, DH, DOUT = 16, 64, 1024
NCORES = 8
GROUP = 4                 # cores per batch group
HL = H // GROUP           # local heads per core = 4
HD = HL * DH              # 256 local head-dim rows
SCALE = 1.0 / float(np.sqrt(np.float32(S)))

SB = 512                  # s-block for transpose-DMA / projection
NSB = S // SB             # 4
QB = 512                  # q-block in attention
NQB = S // QB             # 4
NKT = S // 128            # 16 k-tiles
NIT = D // 128            # 8 d-tiles


def _build_kernel(reps=1):
    nc = bacc.Bacc("TRN2", target_bir_lowering=False, debug=False,
                   num_devices=NCORES)

    xq = nc.dram_tensor("xq", [S, D], BF, kind="ExternalInput").ap()
    xk = nc.dram_tensor("xk", [S, D], BF, kind="ExternalInput").ap()
    xv = nc.dram_tensor("xv", [S, D], BF, kind="ExternalInput").ap()
    wq = nc.dram_tensor("wq", [D, HD], BF, kind="ExternalInput").ap()
    wk = nc.dram_tensor("wk", [D, HD], BF, kind="ExternalInput").ap()
    wv = nc.dram_tensor("wv", [D, HD], BF, kind="ExternalInput").ap()
    wo = nc.dram_tensor("wo", [H * DH, DOUT], BF, kind="ExternalInput").ap()
    gsel = nc.dram_tensor("gsel", [128, 512 // 16], mybir.dt.int16,
                          kind="ExternalInput").ap()
    tsel = nc.dram_tensor("tsel", [128, S // 16], mybir.dt.int16,
                          kind="ExternalInput").ap()
    y = nc.dram_tensor("y", [S // GROUP, DOUT], FP, kind="ExternalOutput").ap()

    groups = [list(range(g * GROUP, (g + 1) * GROUP))
              for g in range(NCORES // GROUP)]

    with tile.TileContext(nc) as tc, ExitStack() as ctx:
        const = ctx.enter_context(tc.tile_pool(name="const", bufs=1))
        xstage = ctx.enter_context(tc.tile_pool(name="xstage", bufs=2))
        xtpose = ctx.enter_context(tc.tile_pool(name="xtpose", bufs=2))
        persist = ctx.enter_context(tc.tile_pool(name="persist", bufs=1))
        ppool = ctx.enter_context(tc.tile_pool(name="ppool", bufs=4))
        opool = ctx.enter_context(tc.tile_pool(name="opool", bufs=4))
        ysb = ctx.enter_context(tc.tile_pool(name="ysb", bufs=2))
        small = ctx.enter_context(tc.tile_pool(name="small", bufs=4))
        # PSUM budget (8 banks): psum_pl [128,2,512]f32 (2 banks) x2 bufs
        # = 4; psum_acc [128,512]f32 (1 bank) x2 = 2; psum_misc (K/Q/V
        # projections + out-projection) [128,512]f32 (1 bank) x2 = 2.
        psum_pl = ctx.enter_context(
            tc.tile_pool(name="psum_pl", bufs=2, space="PSUM"))
        psum_acc = ctx.enter_context(
            tc.tile_pool(name="psum_acc", bufs=2, space="PSUM"))
        psum_misc = ctx.enter_context(
            tc.tile_pool(name="psum_misc", bufs=2, space="PSUM"))
        dram = ctx.enter_context(tc.tile_pool(name="dram", bufs=1, space="DRAM"))

        # Preload the exp activation table while the DMAs run (the first
        # ACTIVATE to a new table set pays ~2.7us).
        warm = const.tile([1, 16], FP)
        nc.vector.memset(warm[:], 0.0)
        warm_o = const.tile([1, 16], FP)
        nc.scalar.activation(warm_o[:], warm[:],
                             mybir.ActivationFunctionType.Exp)

        ident_f = const.tile([128, 128], FP)
        make_identity(nc, ident_f)
        ident = const.tile([128, 128], BF)
        nc.vector.tensor_copy(out=ident[:], in_=ident_f[:])

        # Persistent SBUF tensors (bf16).
        # QT/KT: partition = (h%2)*64 + d, free = (head-pair, s)
        QT = persist.tile([128, 2, S], BF, tag="QT")
        KT = persist.tile([128, 2, S], BF, tag="KT")
        # V: partition = s within s-tile, free = (s-tile, h*65+dv); col h*65+64
        # holds ones so the PV matmul also produces softmax denominators.
        V = persist.tile([128, NKT, HL * 65], BF, tag="V")
        # O^T: partition = (h%2)*64 + dv, free = (head-pair, s)
        OT = persist.tile([128, 2, S], BF, tag="OT")

        v_ones = V.rearrange("p t (h c) -> p t h c", c=65)[:, :, :, 64:65]
        nc.vector.memset(v_ones, 1.0)

        # x^T staging. Three transpose paths run concurrently:
        # - xk: plain DMA + PE transposes (PE is idle early; K gates attention)
        # - xv: X-bar transpose DMAs on the sync/HWDGE queue (~105 GB/s)
        # - xq: SWDGE transpose-gathers on the gpsimd queue
        # xvT: partition = d within d-tile, free = (d-tile, s).
        xvT = persist.tile([128, NIT, S], BF, tag="xvT")
        # xqT: per q-block, free = (d-tile, s within block).
        xqT = persist.tile([128, NQB, NIT, 512], BF, tag="xqT")
        tsel_sb = persist.tile([128, S // 16], mybir.dt.int16, tag="tsel")

        # Weights in SBUF (bf16). wo holds the FULL [H*DH, DOUT] output
        # projection (out-proj runs after the exchange, over all 16 heads).
        wq_sb = persist.tile([128, NIT, HD], BF, tag="wq")
        wk_sb = persist.tile([128, NIT, HD], BF, tag="wk")
        wv_sb = persist.tile([128, NIT, HD], BF, tag="wv")
        wo_sb = persist.tile([128, H * DH // 128, DOUT], BF, tag="wo")

        def load_weight(w_dram, w_t):
            src = w_dram.rearrange("(a p) n -> p a n", p=128)
            nc.sync.dma_start(out=w_t[:], in_=src)

        def emit_transpose_xbar(x_ap, xT, sb):
            """X-bar transpose-DMA s-block sb of x [S, D] into
            xT[:, :, sb*SB:...] (one DMA per d-tile)."""
            for it in range(NIT):
                nc.sync.dma_start_transpose(
                    out=xT[:, it, bass.ts(sb, SB)],
                    in_=x_ap[bass.ts(sb, SB), bass.ts(it, 128)],
                )

        def emit_transpose_gathers(x_ap, b):
            """SWDGE transpose-gather of q-block b of xq into xqT[:, b]."""
            nc.gpsimd.dma_gather(
                out_ap=xqT[:, b, :, :],
                in_ap=x_ap[:],
                idxs_ap=tsel_sb[:, bass.ds(b * 32, 32)],
                num_idxs=512, num_idxs_reg=512,
                elem_size=D, transpose=True)

        def load_transpose_block(x_ap, sb):
            """Plain-DMA s-block sb of x [S, D], PE-transpose it, return an
            SBUF x^T block [128, 8, SB] (partition = d within d-tile,
            free = (d-tile, s))."""
            x_view = x_ap.rearrange("(sb st p) i -> sb p st i", p=128,
                                    st=SB // 128)
            x_sb = xstage.tile([128, SB // 128, D], BF, tag="x_sb")
            for st in range(SB // 128):
                nc.sync.dma_start(out=x_sb[:, st, :], in_=x_view[sb, :, st, :])
            xt = xtpose.tile([128, NIT, SB], BF, tag="xt")
            for it in range(NIT):
                pt = psum_misc.tile([128, SB // 128, 128], BF, tag="misc")
                for st in range(SB // 128):
                    nc.tensor.transpose(
                        pt[:, st, :],
                        x_sb[:, st, bass.ts(it, 128)],
                        ident,
                    )
                nc.vector.tensor_copy(out=xt[:, it, :], in_=pt[:])
            return xt

        def project_k(xt, sb):
            """KT[:, hp, sb*SB:(sb+1)*SB] = (x Wk)^T for both head pairs."""
            for hp in range(2):
                pt = psum_misc.tile([128, 512], FP, tag="misc")
                for it in range(NIT):
                    nc.tensor.matmul(
                        pt[:],
                        wk_sb[:, it, bass.ts(hp, 128)],
                        xt[:, it, :],
                        start=(it == 0), stop=(it == NIT - 1),
                    )
                nc.scalar.copy(out=KT[:, hp, bass.ts(sb, SB)], in_=pt[:])

        def project_q(qb):
            """QT[:, hp, qb*QB:...] = (x Wq)^T from the gathered xqT."""
            for hp in range(2):
                pt = psum_misc.tile([128, 512], FP, tag="misc")
                for it in range(NIT):
                    nc.tensor.matmul(
                        pt[:],
                        wq_sb[:, it, bass.ts(hp, 128)],
                        xqT[:, qb, it, :],
                        start=(it == 0), stop=(it == NIT - 1),
                    )
                nc.vector.tensor_copy(out=QT[:, hp, bass.ts(qb, QB)],
                                      in_=pt[:])

        V4 = V.rearrange("p t (h c) -> p t h c", c=65)

        def project_v(st):
            """V[:, st, :] from xvT (one 128-row s-tile, all 4 heads)."""
            pt = psum_misc.tile([128, 512], FP, tag="misc")
            for it in range(NIT):
                nc.tensor.matmul(
                    pt[:, :HD],
                    xvT[:, it, bass.ts(st, 128)],
                    wv_sb[:, it, :],
                    start=(it == 0), stop=(it == NIT - 1),
                )
            nc.vector.tensor_copy(
                out=V4[:, st, :, 0:64],
                in_=pt[:, :HD].rearrange("p (h c) -> p h c", c=64),
            )

        # Per-(q-block, head-pair) O^T exchange: one 4-way AllGather of the
        # 128-partition head-pair block (128KB in), then one SWDGE gather
        # pulls this core's 128 q-columns of all 4 sources straight into
        # the out-projection operand layout.
        o_send = [[dram.tile([128, QB], BF, tag=f"o_send{qb}_{hp}",
                             name=f"o_send{qb}_{hp}")
                   for hp in range(2)] for qb in range(NQB)]
        o_gath = [[dram.tile([GROUP * 128 * (QB // 128), 128], BF,
                             tag=f"o_gath{qb}_{hp}", name=f"o_gath{qb}_{hp}")
                   for hp in range(2)] for qb in range(NQB)]
        gsel_sb = persist.tile([128, 512 // 16], mybir.dt.int16, tag="gsel")

        def emit_exchange(qb, hp):
            nc.sync.dma_start(out=o_send[qb][hp][:],
                              in_=OT[:, hp, bass.ts(qb, QB)])
            nc.gpsimd.collective_compute(
                "AllGather",
                mybir.AluOpType.bypass,
                replica_groups=groups,
                ins=[o_send[qb][hp].opt()],
                outs=[o_gath[qb][hp].opt()],
            )

        oex_tiles = {}

        def emit_recv(qb, hp):
            # oex[p, j, q]: partition = (h%2)*64 + dv of head 2hp+(h%2) of
            # group-core j, matching wo_sb chunk ch = hp*GROUP + j.
            oex = ysb.tile([128, GROUP, 128], BF, tag=f"oex{hp}",
                           name=f"oex{qb}_{hp}")
            nc.gpsimd.dma_gather(
                out_ap=oex[:],
                in_ap=o_gath[qb][hp][:],
                idxs_ap=gsel_sb[:],
                num_idxs=512,
                num_idxs_reg=512,
                elem_size=128,
            )
            oex_tiles[(qb, hp)] = oex

        def emit_outproj(qb):
            oexs = [oex_tiles.pop((qb, hp)) for hp in range(2)]
            yt = ysb.tile([128, DOUT], FP, tag="yt")
            for ob in range(DOUT // 512):
                py = psum_misc.tile([128, 512], FP, tag="misc")
                for ch in range(2 * GROUP):
                    hp, j = ch // GROUP, ch % GROUP
                    nc.tensor.matmul(
                        py[:],
                        oexs[hp][:, j, :],
                        wo_sb[:, ch, bass.ts(ob, 512)],
                        start=(ch == 0), stop=(ch == 2 * GROUP - 1),
                    )
                nc.vector.tensor_copy(out=yt[:, bass.ts(ob, 512)],
                                      in_=py[:])
            nc.sync.dma_start(out=y[bass.ts(qb, 128), :], in_=yt[:])

        def emit_attention_hp(qb, hp, shadow=None):
            """Attention for head pair hp of q-block qb: row-tiled QK^T
            (both heads concurrently on disjoint PE row groups), one exp
            activation per k-tile covering both heads, PV accumulation
            into one PSUM bank per head. `shadow` maps k-tile index ->
            callback emitted before that iteration (projection work that
            rides in the PE shadow of the ACT-bound loop)."""
            o_acc = [psum_acc.tile([128, 512], FP, tag="acc",
                                   name=f"oacc_{qb}_{hp}_{j}")
                     for j in range(2)]
            p_tiles = [None] * NKT

            def emit_qk(kt):
                pl = psum_pl.tile([128, 2, 512], FP, tag="pl",
                                  name=f"pl_{qb}_{hp}_{kt}")
                for j in range(2):
                    nc.tensor.matmul(
                        pl[:, j, :],
                        KT[64 * j:64 * (j + 1), hp, bass.ts(kt, 128)],
                        QT[64 * j:64 * (j + 1), hp, bass.ts(qb, QB)],
                    )
                p_sb = ppool.tile([128, 2, 512], BF, tag="p_sb",
                                  name=f"p_sb_{qb}_{hp}_{kt}")
                nc.scalar.activation(
                    p_sb[:], pl[:],
                    mybir.ActivationFunctionType.Exp, scale=SCALE,
                )
                p_tiles[kt] = p_sb

            def emit_pv(kt):
                p_sb = p_tiles[kt]
                p_tiles[kt] = None
                for j in range(2):
                    h = 2 * hp + j
                    nc.tensor.matmul(
                        o_acc[j][0:65, :],
                        V[:, kt, h * 65:(h + 1) * 65],
                        p_sb[:, j, :],
                        start=(kt == 0), stop=(kt == NKT - 1),
                        skip_group_check=True,
                    )

            for kt in range(NKT):
                if shadow and kt in shadow:
                    shadow[kt]()
                emit_qk(kt)
                if kt > 0:
                    emit_pv(kt - 1)
            emit_pv(NKT - 1)

            # Normalize O^T by the denominator row (PSUM row 64):
            # reciprocal straight from PSUM, broadcast over 64 partitions
            # on GpSimd, one fused multiply into OT (cast to bf16).
            for j in range(2):
                hr = j * 64
                den = small.tile([1, 512], FP, tag="den")
                nc.vector.tensor_copy(out=den[:], in_=o_acc[j][64:65, :])
                rcp = small.tile([1, 512], FP, tag="rcp")
                nc.vector.reciprocal_approx_fast(rcp[:], den[:])
                rb = opool.tile([64, 512], FP, tag="rb")
                nc.gpsimd.partition_broadcast(rb[:], rcp[:], channels=64)
                nc.vector.tensor_mul(
                    OT[hr:hr + 64, hp, bass.ts(qb, QB)],
                    o_acc[j][0:64, :],
                    rb[:],
                )
            emit_exchange(qb, hp)

        def emit_rep():
            # ---- DMA: three transpose paths in parallel.
            # sync/HWDGE FIFO: tiny index/weight loads, then the xk plain
            # blocks (PE transposes them), then the xv X-bar transposes.
            # gpsimd/SWDGE: the xq transpose-gathers, qb0 first.
            nc.sync.dma_start(out=tsel_sb[:], in_=tsel)
            nc.sync.dma_start(out=gsel_sb[:], in_=gsel)
            load_weight(wk, wk_sb)
            load_weight(wq, wq_sb)
            for b in range(NQB):
                emit_transpose_gathers(xq, b)

            # ---- PE: xk transpose + K projection per s-block; Q-proj(qb0).
            for sb in range(NSB):
                xt = load_transpose_block(xk, sb)
                if sb == 0:
                    load_weight(wv, wv_sb)
                    for vb in range(NSB):
                        emit_transpose_xbar(xv, xvT, vb)
                    load_weight(wo, wo_sb)
                project_k(xt, sb)
            project_q(0)

            # ---- Per q-block attention. V-projection s-tiles ride in the
            # PE shadow of qb0 (V[st] is needed by PV at k-tile st of hp0);
            # Q-proj of qb+1 rides in the hp1 shadow of qb.
            for qb in range(NQB):
                if qb == 0:
                    # project s-tiles 2kt and 2kt+1 at iteration kt: V[st]
                    # is ready before PV consumes it at iteration st+1 and
                    # all 16 s-tiles are emitted by iteration 8.
                    shadow0 = {kt: (lambda st=kt: (project_v(2 * st),
                                                   project_v(2 * st + 1)))
                               for kt in range(8)}
                    emit_attention_hp(qb, 0, shadow=shadow0)
                else:
                    emit_attention_hp(qb, 0)
                if qb >= 1:
                    emit_recv(qb - 1, 0)
                    emit_recv(qb - 1, 1)

                shadow1 = None
                if qb + 1 < NQB:
                    shadow1 = {
                        4: lambda q=qb: project_q(q + 1),
                    }
                emit_attention_hp(qb, 1, shadow=shadow1)
                if qb >= 1:
                    emit_outproj(qb - 1)

            # Tail: only the last q-block's exchange is exposed.
            emit_recv(NQB - 1, 0)
            emit_recv(NQB - 1, 1)
            emit_outproj(NQB - 1)

        for rep in range(reps):
            emit_rep()

    nc.compile()
    return nc


_CACHED_NC = None


def _get_nc():
    global _CACHED_NC
    if _CACHED_NC is None:
        _CACHED_NC = _build_kernel()
    return _CACHED_NC


def _prepare_in_maps(query, key, value, Wq, Wk, Wv, Wo):
    query = np.asarray(query, dtype=np.float32).astype(BF_NP)
    key = np.asarray(key, dtype=np.float32).astype(BF_NP)
    value = np.asarray(value, dtype=np.float32).astype(BF_NP)
    Wq = np.asarray(Wq, dtype=np.float32).astype(BF_NP)
    Wk = np.asarray(Wk, dtype=np.float32).astype(BF_NP)
    Wv = np.asarray(Wv, dtype=np.float32).astype(BF_NP)
    Wo = np.asarray(Wo, dtype=np.float32).astype(BF_NP)

    in_maps = []
    for c in range(NCORES):
        b, g = c // GROUP, c % GROUP
        hs = slice(g * HL, (g + 1) * HL)
        # Gather indices: output position i = j*128 + p (j = source core,
        # p = partition) pulls o_gath row j*512 + p*4 + g; entry i is
        # stored at [i % 16, i // 16], tiled across the 8 partition groups.
        i = np.arange(512)
        lin = ((i // 128) * 512 + (i % 128) * 4 + g).astype(np.int16)
        g16 = np.zeros((16, 32), dtype=np.int16)
        g16[i % 16, i // 16] = lin
        gsel_arr = np.tile(g16, (8, 1))
        t16 = np.zeros((16, S // 16), dtype=np.int16)
        t16[np.arange(S) % 16, np.arange(S) // 16] = np.arange(S, dtype=np.int16)
        tsel_arr = np.tile(t16, (8, 1))
        in_maps.append({
            "gsel": gsel_arr,
            "tsel": tsel_arr,
            "xq": np.ascontiguousarray(query[b]),
            "xk": np.ascontiguousarray(key[b]),
            "xv": np.ascontiguousarray(value[b]),
            "wq": np.ascontiguousarray(
                Wq[hs].transpose(1, 0, 2).reshape(D, HD)),
            "wk": np.ascontiguousarray(
                Wk[hs].transpose(1, 0, 2).reshape(D, HD)),
            "wv": np.ascontiguousarray(
                Wv[hs].transpose(1, 0, 2).reshape(D, HD)),
            # wo chunk ch = hp*GROUP + j holds rows (j2, dv) of global
            # head 4j + 2hp + j2, matching the AllToAll receive layout.
            "wo": np.ascontiguousarray(
                Wo.reshape(GROUP, 2, 2, DH, DOUT).transpose(1, 0, 2, 3, 4)
                .reshape(H * DH, DOUT)),
        })
    return in_maps


def _assemble(results):
    out = np.empty((B, S, DOUT), dtype=np.float32)
    for c in range(NCORES):
        b, g = c // GROUP, c % GROUP
        yc = results[c]["y"]  # [512, DOUT]: row qb*128+r = batch row qb*512+g*128+r
        for qb in range(NQB):
            out[b, qb * QB + g * 128: qb * QB + (g + 1) * 128, :] = \
                yc[qb * 128:(qb + 1) * 128, :]
    return out


def kernel(query, key, value, mask, Wq, Wk, Wv, Wo):
    nc = _get_nc()
    in_maps = _prepare_in_maps(query, key, value, Wq, Wk, Wv, Wo)
    results = run_bass_kernel_spmd(nc, in_maps, list(range(NCORES))).results
    return _assemble(results)


# revision 27
# speedup vs baseline: 1.2739x; 1.2438x over previous
"""Multi-head attention on 8 Trainium2 NeuronCores.

Sharding: data-parallel over batch (2 groups of 4 cores), tensor-parallel
over heads within each group (4 heads/core). Per (q-block, head-pair) a
4-way AllToAll exchanges exactly the O^T slice each peer needs for its
128 output rows; each core then runs the full output projection (all 16
heads) for its 512-row slice of its batch's output.

Layout/engine choices:
- x^T for q/k/v lands in SBUF via HWDGE transpose DMAs (X-bar 256B-tile
  transpose on the sync queue) — no PE transposes, no SWDGE gathers.
- Attention inner loop iterates (q-block, head-PAIR, k-tile): the two
  heads of a pair live at partitions 0-63 / 64-127 of QT/KT, so their
  K=64 QK^T matmuls target disjoint PE row groups and run concurrently
  (row tiling). One ScalarE exp covers both heads' logits [128, 1024]
  per k-tile; ScalarE is the pipeline metronome (~1.15us/k-tile).
- Only K-proj and Q-proj(qb0) run before attention; V-projection and
  later Q-projections execute in the PE shadow of the ACT-bound loop.
- V carries a ones column per head so PV also accumulates the softmax
  denominators (PSUM row 64).

Problem shapes (hardcoded): B=2, S=2048, D=1024, H=16, DQK=DV=64, DOUT=1024.
mask is all-ones in this problem, so it contributes 0 to the logits and is
ignored.
"""

import numpy as np
import ml_dtypes
from contextlib import ExitStack

import concourse.bass as bass
import concourse.bacc as bacc
import concourse.tile as tile
import concourse.mybir as mybir
from concourse.bass_utils import run_bass_kernel_spmd
from concourse.masks import make_identity

FP = mybir.dt.float32
BF = mybir.dt.bfloat16
BF_NP = ml_dtypes.bfloat16

B, S, D = 2, 2048, 1024
H, DH, DOUT = 16, 64, 1024
NCORES = 8
GROUP = 4                 # cores per batch group
HL = H // GROUP           # local heads per core = 4
HD = HL * DH              # 256 local head-dim rows
SCALE = 1.0 / float(np.sqrt(np.float32(S)))

SB = 512                  # s-block for transpose-DMA / projection
NSB = S // SB             # 4
QB = 512                  # q-block in attention
NQB = S // QB             # 4
NKT = S // 128            # 16 k-tiles
NIT = D // 128            # 8 d-tiles


def _build_kernel(reps=1):
    nc = bacc.Bacc("TRN2", target_bir_lowering=False, debug=False,
                   num_devices=NCORES)

    xq = nc.dram_tensor("xq", [S, D], BF, kind="ExternalInput").ap()
    xk = nc.dram_tensor("xk", [S, D], BF, kind="ExternalInput").ap()
    xv = nc.dram_tensor("xv", [S, D], BF, kind="ExternalInput").ap()
    wq = nc.dram_tensor("wq", [D, HD], BF, kind="ExternalInput").ap()
    wk = nc.dram_tensor("wk", [D, HD], BF, kind="ExternalInput").ap()
    wv = nc.dram_tensor("wv", [D, HD], BF, kind="ExternalInput").ap()
    wo = nc.dram_tensor("wo", [H * DH, DOUT], BF, kind="ExternalInput").ap()
    gsel = nc.dram_tensor("gsel", [128, 512 // 16], mybir.dt.int16,
                          kind="ExternalInput").ap()
    tsel = nc.dram_tensor("tsel", [128, S // 16], mybir.dt.int16,
                          kind="ExternalInput").ap()
    y = nc.dram_tensor("y", [S // GROUP, DOUT], FP, kind="ExternalOutput").ap()

    groups = [list(range(g * GROUP, (g + 1) * GROUP))
              for g in range(NCORES // GROUP)]

    with tile.TileContext(nc) as tc, ExitStack() as ctx:
        const = ctx.enter_context(tc.tile_pool(name="const", bufs=1))
        xstage = ctx.enter_context(tc.tile_pool(name="xstage", bufs=4))
        xtpose = ctx.enter_context(tc.tile_pool(name="xtpose", bufs=2))
        persist = ctx.enter_context(tc.tile_pool(name="persist", bufs=1))
        ppool = ctx.enter_context(tc.tile_pool(name="ppool", bufs=4))
        opool = ctx.enter_context(tc.tile_pool(name="opool", bufs=2))
        ysb = ctx.enter_context(tc.tile_pool(name="ysb", bufs=2))
        small = ctx.enter_context(tc.tile_pool(name="small", bufs=2))
        # PSUM budget (8 banks): psum_pl [128,2,512]f32 (2 banks) x2 bufs
        # = 4; psum_acc [128,512]f32 (1 bank) x2 = 2; psum_misc (K/Q/V
        # projections + out-projection) [128,512]f32 (1 bank) x2 = 2.
        psum_pl = ctx.enter_context(
            tc.tile_pool(name="psum_pl", bufs=2, space="PSUM"))
        psum_acc = ctx.enter_context(
            tc.tile_pool(name="psum_acc", bufs=2, space="PSUM"))
        psum_misc = ctx.enter_context(
            tc.tile_pool(name="psum_misc", bufs=2, space="PSUM"))
        dram = ctx.enter_context(tc.tile_pool(name="dram", bufs=1, space="DRAM"))

        # Preload the exp activation table while the DMAs run (the first
        # ACTIVATE to a new table set pays ~2.7us).
        warm = const.tile([1, 16], FP)
        nc.vector.memset(warm[:], 0.0)
        warm_o = const.tile([1, 16], FP)
        nc.scalar.activation(warm_o[:], warm[:],
                             mybir.ActivationFunctionType.Exp)

        ident_f = const.tile([128, 128], FP)
        make_identity(nc, ident_f)
        ident = const.tile([128, 128], BF)
        nc.vector.tensor_copy(out=ident[:], in_=ident_f[:])

        # Persistent SBUF tensors (bf16).
        # QT/KT: partition = (h%2)*64 + d, free = (head-pair, s)
        QT = persist.tile([128, 2, S], BF, tag="QT")
        KT = persist.tile([128, 2, S], BF, tag="KT")
        # V: partition = s within s-tile, free = (s-tile, h*65+dv); col h*65+64
        # holds ones so the PV matmul also produces softmax denominators.
        V = persist.tile([128, NKT, HL * 65], BF, tag="V")
        # O^T: partition = (h%2)*64 + dv, free = (head-pair, s)
        OT = persist.tile([128, 2, S], BF, tag="OT")

        v_ones = V.rearrange("p t (h c) -> p t h c", c=65)[:, :, :, 64:65]
        nc.vector.memset(v_ones, 1.0)

        # x^T staging. Three transpose paths run concurrently:
        # - xk: plain DMA + PE transposes (PE is idle early; K gates attention)
        # - xv: X-bar transpose DMAs on the sync/HWDGE queue (~105 GB/s)
        # - xq: SWDGE transpose-gathers on the gpsimd queue
        # xvT: partition = d within d-tile, free = (d-tile, s).
        xvT = persist.tile([128, NIT, S], BF, tag="xvT")
        # xqT: per q-block, free = (d-tile, s within block).
        xqT = persist.tile([128, NQB, NIT, 512], BF, tag="xqT")
        tsel_sb = persist.tile([128, S // 16], mybir.dt.int16, tag="tsel")

        # Weights in SBUF (bf16). wo holds the FULL [H*DH, DOUT] output
        # projection (out-proj runs after the exchange, over all 16 heads).
        wq_sb = persist.tile([128, NIT, HD], BF, tag="wq")
        wk_sb = persist.tile([128, NIT, HD], BF, tag="wk")
        wv_sb = persist.tile([128, NIT, HD], BF, tag="wv")
        wo_sb = persist.tile([128, H * DH // 128, DOUT], BF, tag="wo")

        def load_weight(w_dram, w_t):
            src = w_dram.rearrange("(a p) n -> p a n", p=128)
            nc.sync.dma_start(out=w_t[:], in_=src)

        def emit_transpose_xbar(x_ap, xT, sb):
            """X-bar transpose-DMA s-block sb of x [S, D] into
            xT[:, :, sb*SB:...] (one DMA per d-tile)."""
            for it in range(NIT):
                nc.sync.dma_start_transpose(
                    out=xT[:, it, bass.ts(sb, SB)],
                    in_=x_ap[bass.ts(sb, SB), bass.ts(it, 128)],
                )

        def emit_transpose_gathers(x_ap, b):
            """SWDGE transpose-gather of q-block b of xq into xqT[:, b]."""
            nc.gpsimd.dma_gather(
                out_ap=xqT[:, b, :, :],
                in_ap=x_ap[:],
                idxs_ap=tsel_sb[:, bass.ds(b * 32, 32)],
                num_idxs=512, num_idxs_reg=512,
                elem_size=D, transpose=True)

        def stage_block(x_ap, sb):
            """Plain-DMA s-block sb of x [S, D] into an SBUF staging tile."""
            x_view = x_ap.rearrange("(sb st p) i -> sb p st i", p=128,
                                    st=SB // 128)
            x_sb = xstage.tile([128, SB // 128, D], BF, tag="x_sb",
                               name=f"x_sb{sb}")
            for st in range(SB // 128):
                nc.sync.dma_start(out=x_sb[:, st, :], in_=x_view[sb, :, st, :])
            return x_sb

        def transpose_block(x_sb):
            """PE-transpose a staged block, return SBUF x^T block
            [128, 8, SB] (partition = d within d-tile, free = (d-tile, s))."""
            xt = xtpose.tile([128, NIT, SB], BF, tag="xt")
            for it in range(NIT):
                pt = psum_misc.tile([128, SB // 128, 128], BF, tag="misc")
                for st in range(SB // 128):
                    nc.tensor.transpose(
                        pt[:, st, :],
                        x_sb[:, st, bass.ts(it, 128)],
                        ident,
                    )
                nc.vector.tensor_copy(out=xt[:, it, :], in_=pt[:])
            return xt

        def project_k(xt, sb):
            """KT[:, hp, sb*SB:(sb+1)*SB] = (x Wk)^T for both head pairs."""
            for hp in range(2):
                pt = psum_misc.tile([128, 512], FP, tag="misc")
                for it in range(NIT):
                    nc.tensor.matmul(
                        pt[:],
                        wk_sb[:, it, bass.ts(hp, 128)],
                        xt[:, it, :],
                        start=(it == 0), stop=(it == NIT - 1),
                    )
                nc.scalar.copy(out=KT[:, hp, bass.ts(sb, SB)], in_=pt[:])

        def project_q(qb):
            """QT[:, hp, qb*QB:...] = (x Wq)^T from the gathered xqT."""
            for hp in range(2):
                pt = psum_misc.tile([128, 512], FP, tag="misc")
                for it in range(NIT):
                    nc.tensor.matmul(
                        pt[:],
                        wq_sb[:, it, bass.ts(hp, 128)],
                        xqT[:, qb, it, :],
                        start=(it == 0), stop=(it == NIT - 1),
                    )
                nc.vector.tensor_copy(out=QT[:, hp, bass.ts(qb, QB)],
                                      in_=pt[:])

        V4 = V.rearrange("p t (h c) -> p t h c", c=65)

        def project_v(st):
            """V[:, st, :] from xvT (one 128-row s-tile, all 4 heads)."""
            pt = psum_misc.tile([128, 512], FP, tag="misc")
            for it in range(NIT):
                nc.tensor.matmul(
                    pt[:, :HD],
                    xvT[:, it, bass.ts(st, 128)],
                    wv_sb[:, it, :],
                    start=(it == 0), stop=(it == NIT - 1),
                )
            nc.vector.tensor_copy(
                out=V4[:, st, :, 0:64],
                in_=pt[:, :HD].rearrange("p (h c) -> p h c", c=64),
            )

        # Per-(q-block, head-pair) O^T exchange: one 4-way AllGather of the
        # 128-partition head-pair block (128KB in), then one SWDGE gather
        # pulls this core's 128 q-columns of all 4 sources straight into
        # the out-projection operand layout.
        o_send = [[dram.tile([128, QB], BF, tag=f"o_send{qb}_{hp}",
                             name=f"o_send{qb}_{hp}")
                   for hp in range(2)] for qb in range(NQB)]
        o_gath = [[dram.tile([GROUP * 128 * (QB // 128), 128], BF,
                             tag=f"o_gath{qb}_{hp}", name=f"o_gath{qb}_{hp}")
                   for hp in range(2)] for qb in range(NQB)]
        gsel_sb = persist.tile([128, 512 // 16], mybir.dt.int16, tag="gsel")

        def emit_exchange(qb, hp):
            nc.sync.dma_start(out=o_send[qb][hp][:],
                              in_=OT[:, hp, bass.ts(qb, QB)])
            nc.gpsimd.collective_compute(
                "AllGather",
                mybir.AluOpType.bypass,
                replica_groups=groups,
                ins=[o_send[qb][hp].opt()],
                outs=[o_gath[qb][hp].opt()],
            )

        oex_tiles = {}

        def emit_recv(qb, hp):
            # oex[p, j, q]: partition = (h%2)*64 + dv of head 2hp+(h%2) of
            # group-core j, matching wo_sb chunk ch = hp*GROUP + j.
            oex = ysb.tile([128, GROUP, 128], BF, tag=f"oex{hp}",
                           name=f"oex{qb}_{hp}")
            nc.gpsimd.dma_gather(
                out_ap=oex[:],
                in_ap=o_gath[qb][hp][:],
                idxs_ap=gsel_sb[:],
                num_idxs=512,
                num_idxs_reg=512,
                elem_size=128,
            )
            oex_tiles[(qb, hp)] = oex

        yt_tiles = {}

        def emit_outproj_part(qb, ob):
            """One 512-column half of the out-projection of q-block qb;
            split so each half can ride the PE shadow separately."""
            if ob == 0:
                yt_tiles[qb] = ysb.tile([128, DOUT], FP, tag="yt",
                                        name=f"yt{qb}")
            yt = yt_tiles[qb]
            py = psum_misc.tile([128, 512], FP, tag="misc")
            for ch in range(2 * GROUP):
                hp, j = ch // GROUP, ch % GROUP
                nc.tensor.matmul(
                    py[:],
                    oex_tiles[(qb, hp)][:, j, :],
                    wo_sb[:, ch, bass.ts(ob, 512)],
                    start=(ch == 0), stop=(ch == 2 * GROUP - 1),
                )
            nc.vector.tensor_copy(out=yt[:, bass.ts(ob, 512)], in_=py[:])
            if ob == DOUT // 512 - 1:
                for hp in range(2):
                    oex_tiles.pop((qb, hp))
                yt_tiles.pop(qb)
                nc.sync.dma_start(out=y[bass.ts(qb, 128), :], in_=yt[:])

        def emit_attention_hp(qb, hp, shadow=None):
            """Attention for head pair hp of q-block qb: row-tiled QK^T
            (both heads concurrently on disjoint PE row groups), one exp
            activation per k-tile covering both heads, PV accumulation
            into one PSUM bank per head. `shadow` maps k-tile index ->
            callback emitted before that iteration (projection work that
            rides in the PE shadow of the ACT-bound loop)."""
            o_acc = [psum_acc.tile([128, 512], FP, tag="acc",
                                   name=f"oacc_{qb}_{hp}_{j}")
                     for j in range(2)]
            p_tiles = [None] * NKT

            def emit_qk(kt):
                pl = psum_pl.tile([128, 2, 512], FP, tag="pl",
                                  name=f"pl_{qb}_{hp}_{kt}")
                for j in range(2):
                    nc.tensor.matmul(
                        pl[:, j, :],
                        KT[64 * j:64 * (j + 1), hp, bass.ts(kt, 128)],
                        QT[64 * j:64 * (j + 1), hp, bass.ts(qb, QB)],
                    )
                p_sb = ppool.tile([128, 2, 512], BF, tag="p_sb",
                                  name=f"p_sb_{qb}_{hp}_{kt}")
                nc.scalar.activation(
                    p_sb[:], pl[:],
                    mybir.ActivationFunctionType.Exp, scale=SCALE,
                )
                p_tiles[kt] = p_sb

            def emit_pv(kt):
                p_sb = p_tiles[kt]
                p_tiles[kt] = None
                for j in range(2):
                    h = 2 * hp + j
                    nc.tensor.matmul(
                        o_acc[j][0:65, :],
                        V[:, kt, h * 65:(h + 1) * 65],
                        p_sb[:, j, :],
                        start=(kt == 0), stop=(kt == NKT - 1),
                        skip_group_check=True,
                    )

            for kt in range(NKT):
                if shadow and kt in shadow:
                    shadow[kt]()
                emit_qk(kt)
                if kt > 0:
                    emit_pv(kt - 1)
            emit_pv(NKT - 1)

            # Normalize O^T by the denominator row (PSUM row 64):
            # reciprocal straight from PSUM, broadcast over 64 partitions
            # on GpSimd, one fused multiply into OT (cast to bf16).
            for j in range(2):
                hr = j * 64
                den = small.tile([1, 512], FP, tag="den")
                nc.vector.tensor_copy(out=den[:], in_=o_acc[j][64:65, :])
                rcp = small.tile([1, 512], FP, tag="rcp")
                nc.vector.reciprocal_approx_fast(rcp[:], den[:])
                rb = opool.tile([64, 512], FP, tag="rb")
                nc.gpsimd.partition_broadcast(rb[:], rcp[:], channels=64)
                nc.vector.tensor_mul(
                    OT[hr:hr + 64, hp, bass.ts(qb, QB)],
                    o_acc[j][0:64, :],
                    rb[:],
                )
            emit_exchange(qb, hp)

        def emit_rep():
            # ---- DMA: three transpose paths in parallel.
            # sync/HWDGE FIFO: tiny index/weight loads, ALL xk plain blocks
            # (PE transposes them), then the xv X-bar transposes and wo.
            # gpsimd/SWDGE: the xq transpose-gathers, qb0 first.
            nc.sync.dma_start(out=tsel_sb[:], in_=tsel)
            nc.sync.dma_start(out=gsel_sb[:], in_=gsel)
            load_weight(wk, wk_sb)
            load_weight(wq, wq_sb)
            for b in range(NQB):
                emit_transpose_gathers(xq, b)
            x_sbs = [stage_block(xk, sb) for sb in range(NSB)]
            load_weight(wv, wv_sb)
            for vb in range(NSB):
                emit_transpose_xbar(xv, xvT, vb)
            load_weight(wo, wo_sb)

            # ---- PE: xk transpose + K projection per s-block; Q-proj(qb0).
            for sb in range(NSB):
                project_k(transpose_block(x_sbs[sb]), sb)
            project_q(0)

            # ---- Per q-block attention. V-projection s-tiles ride in the
            # PE shadow of qb0 (V[st] is needed by PV at k-tile st+2 of
            # hp0); Q-proj of qb+1 rides in the hp1 shadow of qb; the
            # out-projection halves of qb-2 ride in the hp0 shadow of qb
            # (the AllGather takes ~25us to land, so its consumers run two
            # q-blocks after the trigger).
            for qb in range(NQB):
                shadow0 = {}
                if qb == 0:
                    shadow0 = {kt: (lambda st=kt: (project_v(2 * st),
                                                   project_v(2 * st + 1)))
                               for kt in range(8)}
                emit_attention_hp(qb, 0, shadow=shadow0)
                if qb >= 2:
                    emit_outproj_part(qb - 2, 0)

                shadow1 = None
                if qb + 1 < NQB:
                    shadow1 = {
                        4: lambda q=qb: project_q(q + 1),
                    }
                emit_attention_hp(qb, 1, shadow=shadow1)
                if qb >= 2:
                    emit_outproj_part(qb - 2, 1)
                if qb >= 1:
                    emit_recv(qb - 1, 0)
                    emit_recv(qb - 1, 1)

            # Tail: the last two q-blocks' out-projections; only qb3's
            # exchange latency is exposed.
            for ob in range(2):
                emit_outproj_part(NQB - 2, ob)
            emit_recv(NQB - 1, 0)
            emit_recv(NQB - 1, 1)
            for ob in range(2):
                emit_outproj_part(NQB - 1, ob)

        for rep in range(reps):
            emit_rep()

    nc.compile()
    return nc


_CACHED_NC = None


def _get_nc():
    global _CACHED_NC
    if _CACHED_NC is None:
        _CACHED_NC = _build_kernel()
    return _CACHED_NC


def _prepare_in_maps(query, key, value, Wq, Wk, Wv, Wo):
    query = np.asarray(query, dtype=np.float32).astype(BF_NP)
    key = np.asarray(key, dtype=np.float32).astype(BF_NP)
    value = np.asarray(value, dtype=np.float32).astype(BF_NP)
    Wq = np.asarray(Wq, dtype=np.float32).astype(BF_NP)
    Wk = np.asarray(Wk, dtype=np.float32).astype(BF_NP)
    Wv = np.asarray(Wv, dtype=np.float32).astype(BF_NP)
    Wo = np.asarray(Wo, dtype=np.float32).astype(BF_NP)

    in_maps = []
    for c in range(NCORES):
        b, g = c // GROUP, c % GROUP
        hs = slice(g * HL, (g + 1) * HL)
        # Gather indices: output position i = j*128 + p (j = source core,
        # p = partition) pulls o_gath row j*512 + p*4 + g; entry i is
        # stored at [i % 16, i // 16], tiled across the 8 partition groups.
        i = np.arange(512)
        lin = ((i // 128) * 512 + (i % 128) * 4 + g).astype(np.int16)
        g16 = np.zeros((16, 32), dtype=np.int16)
        g16[i % 16, i // 16] = lin
        gsel_arr = np.tile(g16, (8, 1))
        t16 = np.zeros((16, S // 16), dtype=np.int16)
        t16[np.arange(S) % 16, np.arange(S) // 16] = np.arange(S, dtype=np.int16)
        tsel_arr = np.tile(t16, (8, 1))
        in_maps.append({
            "gsel": gsel_arr,
            "tsel": tsel_arr,
            "xq": np.ascontiguousarray(query[b]),
            "xk": np.ascontiguousarray(key[b]),
            "xv": np.ascontiguousarray(value[b]),
            "wq": np.ascontiguousarray(
                Wq[hs].transpose(1, 0, 2).reshape(D, HD)),
            "wk": np.ascontiguousarray(
                Wk[hs].transpose(1, 0, 2).reshape(D, HD)),
            "wv": np.ascontiguousarray(
                Wv[hs].transpose(1, 0, 2).reshape(D, HD)),
            # wo chunk ch = hp*GROUP + j holds rows (j2, dv) of global
            # head 4j + 2hp + j2, matching the AllToAll receive layout.
            "wo": np.ascontiguousarray(
                Wo.reshape(GROUP, 2, 2, DH, DOUT).transpose(1, 0, 2, 3, 4)
                .reshape(H * DH, DOUT)),
        })
    return in_maps


def _assemble(results):
    out = np.empty((B, S, DOUT), dtype=np.float32)
    for c in range(NCORES):
        b, g = c // GROUP, c % GROUP
        yc = results[c]["y"]  # [512, DOUT]: row qb*128+r = batch row qb*512+g*128+r
        for qb in range(NQB):
            out[b, qb * QB + g * 128: qb * QB + (g + 1) * 128, :] = \
                yc[qb * 128:(qb + 1) * 128, :]
    return out


def kernel(query, key, value, mask, Wq, Wk, Wv, Wo):
    nc = _get_nc()
    in_maps = _prepare_in_maps(query, key, value, Wq, Wk, Wv, Wo)
    results = run_bass_kernel_spmd(nc, in_maps, list(range(NCORES))).results
    return _assemble(results)
